# revision 1
# baseline (speedup 1.0000x reference)
# Trainium2 Bass kernel for EvidenceRetriever (cosine-sim retrieval + top-8).
#
# score[t, s] = <t_hat, s_hat> + 0.1 * importance[s]   (t_hat/s_hat L2-normalized)
# outputs: top-8 indices (int32), top-8 scores (f32, desc), softmax over the 8.
#
# Sharding: data-parallel over target rows — 8 cores x 2048 rows each;
# source_edge_feats + importance replicated. No cross-core communication.
#
# Per-core pipeline (two-precision candidate/rescore design):
#   The PE's fast fp32 mode (float32r, 1 cycle/row vs 4 for true fp32) carries
#   ~1.4e-4 relative error — enough to flip ~2.5% of exact top-8 sets, but the
#   exact top-8 always sits inside the f32r top-16 (>=5x margin on this data).
#   So: f32r matmuls generate top-16 candidates per row, then the 16 candidate
#   dot products are recomputed in exact fp32 on the vector engine.
#
#   phase 0: normalize targets (fp32) + PE-transpose into the f32r tile.
#   phase A: per source chunk: normalize+transpose chunk into f32r tiles,
#            matmul (4 K-tiles + K=1 bias matmul vs 0.1*importance) into PSUM,
#            ACT-copy to SBUF strip, DVE max/max_index -> per-chunk top-8
#            (values + global indices, f32).
#   stage 2: per target tile: top-16 of the compact buffer (max, match_replace,
#            max); candidate ids via fused eq-match+accum.
#   phase B: per candidate rank: indirect-DMA gather source rows (and their
#            importance) by id, renormalize, fused exact dot (+bias) on DVE
#            -> exact scores; final top-8 by exact score, ids via unique
#            compact positions, softmax on ACT.
import os
from contextlib import ExitStack

import numpy as np

import concourse.bass as bass
import concourse.tile as tile
from concourse import bacc, mybir
from concourse.bass_utils import run_bass_kernel_spmd
from concourse.masks import make_identity

F32 = mybir.dt.float32
F32R = mybir.dt.float32r
U32 = mybir.dt.uint32
I32 = mybir.dt.int32
AF = mybir.ActivationFunctionType
ALU = mybir.AluOpType

N_CORES = 8
E_T, E_S, FDIM = 16384, 32768, 512
T_LOCAL = E_T // N_CORES
K = 8
NCAND = 16            # f32r candidates per row, rescored exactly
W_IMPORTANCE = 0.1
CHUNK = 1024          # source columns per strip (multiple of 512)

LAST_RESULTS = None


def build_program(t_local=T_LOCAL, e_s=E_S, fdim=FDIM, chunk=CHUNK,
                  ablate=(), repeat=1):
    assert t_local % 128 == 0 and e_s % chunk == 0 and chunk % 512 == 0
    n_tt = t_local // 128
    n_sc = e_s // chunk
    n_kt = fdim // 128
    n_seg = chunk // 512
    cw = n_sc * K                  # compact width per target tile
    ncand = min(NCAND, cw)

    nc = bacc.Bacc(None, target_bir_lowering=False, debug=False)
    t_in = nc.dram_tensor("t", [t_local, fdim], F32, kind="ExternalInput")
    s_in = nc.dram_tensor("s", [e_s, fdim], F32, kind="ExternalInput")
    imp_in = nc.dram_tensor("imp", [1, e_s], F32, kind="ExternalInput")
    impt_in = nc.dram_tensor("impt", [e_s, 1], F32, kind="ExternalInput")
    idx_out = nc.dram_tensor("idx", [t_local, K], I32, kind="ExternalOutput")
    score_out = nc.dram_tensor("score", [t_local, K], F32, kind="ExternalOutput")
    alpha_out = nc.dram_tensor("alpha", [t_local, K], F32, kind="ExternalOutput")

    with tile.TileContext(nc) as tc:
        with ExitStack() as ctx:
            const = ctx.enter_context(tc.tile_pool(name="const", bufs=1))
            prep = ctx.enter_context(tc.tile_pool(name="prep", bufs=6))
            tpsum = ctx.enter_context(
                tc.tile_pool(name="tpsum", bufs=2, space="PSUM"))
            sT_pool = ctx.enter_context(tc.tile_pool(name="sT", bufs=2))
            mm_psum = ctx.enter_context(
                tc.tile_pool(name="mm", bufs=3, space="PSUM"))
            small = ctx.enter_context(tc.tile_pool(name="small", bufs=4))
            fin = ctx.enter_context(tc.tile_pool(name="fin", bufs=2))
            gat = ctx.enter_context(tc.tile_pool(name="gat", bufs=4))

            identity = const.tile([128, 128], F32)
            make_identity(nc, identity[:])
            iota_cw_i = const.tile([128, cw], I32)
            nc.gpsimd.iota(iota_cw_i[:], pattern=[[1, cw]], base=0,
                           channel_multiplier=0)
            iota_cw = const.tile([128, cw], F32)
            nc.vector.tensor_copy(iota_cw[:], iota_cw_i[:])
            iota_nc_i = const.tile([128, ncand], I32)
            nc.gpsimd.iota(iota_nc_i[:], pattern=[[1, ncand]], base=0,
                           channel_multiplier=0)
            iota_nc = const.tile([128, ncand], F32)
            nc.vector.tensor_copy(iota_nc[:], iota_nc_i[:])
            wimp_f = const.tile([1, 128], F32)
            nc.vector.memset(wimp_f[:], W_IMPORTANCE)
            wimp = const.tile([1, 128], F32R)
            nc.scalar.copy(wimp[:], wimp_f[:])

            # Residents: transposed f32r targets and the per-tile compact
            # candidate buffers.
            tT_all = const.tile([128, n_kt * t_local], F32R)
            cvals = [const.tile([128, cw], F32, name=f"cvals{i}")
                     for i in range(n_tt)]
            cidx = [const.tile([128, cw], F32, name=f"cidx{i}")
                    for i in range(n_tt)]

            def normalize_tile(dstT_all, blk_w, dst_col, src_rows, norm):
                """Load 128 rows, L2-normalize into `norm` (fp32), transpose
                128x128 blocks into dstT_all (one wide f32r tile whose k-tile
                j occupies columns [j*blk_w, (j+1)*blk_w))."""
                raw = prep.tile([128, fdim], F32, tag="raw")
                nc.sync.dma_start(raw[:], src_rows)
                sq = prep.tile([128, fdim], F32, tag="sq")
                ss = prep.tile([128, 1], F32, tag="ss")
                nc.scalar.activation(sq[:], raw[:], AF.Square, accum_out=ss[:])
                nrm = prep.tile([128, 1], F32, tag="nrm")
                nc.scalar.sqrt(nrm[:], ss[:])
                inv = prep.tile([128, 1], F32, tag="inv")
                nc.vector.reciprocal(inv[:], nrm[:])
                nc.vector.tensor_scalar_mul(norm[:], raw[:], inv[:])
                pt = tpsum.tile([128, n_kt * 128], F32)
                for j in range(n_kt):
                    nc.tensor.transpose(
                        pt[:, j * 128:(j + 1) * 128],
                        norm[:, j * 128:(j + 1) * 128], identity[:])
                dst3 = dstT_all.rearrange(
                    "p (j c) -> p j c", j=n_kt)[:, :, dst_col:dst_col + 128]
                nc.scalar.copy(
                    dst3, pt[:].rearrange("p (j c) -> p j c", j=n_kt))

            # Phase 0: targets
            for tt in range(n_tt):
                t0norm = prep.tile([128, fdim], F32, tag="snorm")
                normalize_tile(tT_all, t_local, tt * 128,
                               t_in.ap()[tt * 128:(tt + 1) * 128, :],
                               t0norm)

            # Phase A: stream source chunks, collect per-chunk top-8
            # (repeat>1 re-emits the whole compute for timing-slope runs)
            for _rep, sc in [(r, c) for r in range(repeat)
                             for c in range(n_sc)]:
                sT_all = sT_pool.tile([128, n_kt * chunk], F32R, tag="sT")
                for r in range(chunk // 128):
                    row0 = sc * chunk + r * 128
                    snorm = prep.tile([128, fdim], F32, tag="snorm")
                    normalize_tile(sT_all, chunk, r * 128,
                                   s_in.ap()[row0:row0 + 128, :], snorm)
                bias_r = [small.tile([1, 512], F32R, name=f"biasr{n}",
                                     tag=f"biasr{n}") for n in range(n_seg)]
                for n in range(n_seg):
                    c0 = sc * chunk + n * 512
                    bias_f = small.tile([1, 512], F32, tag="biasf")
                    nc.sync.dma_start(bias_f[:], imp_in.ap()[0:1, c0:c0 + 512])
                    nc.scalar.copy(bias_r[n][:], bias_f[:])
                for tt in range(n_tt):
                    ps = mm_psum.tile([128, chunk], F32, tag="ps")
                    for n in range(n_seg):
                        seg = ps[:, n * 512:(n + 1) * 512]
                        for j in range(n_kt):
                            nc.tensor.matmul(
                                seg,
                                tT_all[:, j * t_local + tt * 128:
                                       j * t_local + (tt + 1) * 128],
                                sT_all[:, j * chunk + n * 512:
                                       j * chunk + (n + 1) * 512],
                                start=(j == 0), stop=False)
                        if "bias_mm" not in ablate:
                            nc.tensor.matmul(seg, wimp[:], bias_r[n][:],
                                             start=False, stop=True)
                    cv = cvals[tt][:, sc * K:(sc + 1) * K]
                    if "max" not in ablate:
                        nc.vector.max(cv, ps[:])
                    else:
                        nc.vector.memset(cv, 0.0)
                    if "maxidx" not in ablate:
                        lidx = small.tile([128, K], U32, tag="lidx")
                        nc.vector.max_index(lidx[:], cv, ps[:])
                        nc.vector.tensor_scalar(
                            cidx[tt][:, sc * K:(sc + 1) * K], lidx[:],
                            float(sc * chunk), None, op0=ALU.add)
                    else:
                        nc.vector.memset(
                            cidx[tt][:, sc * K:(sc + 1) * K], 0.0)

            # Stage 2 + Phase B per target tile
            for _rep, tt in [(r, i) for r in range(repeat)
                             for i in range(n_tt)]:
                # top-16 candidate values + their compact positions
                # (positions are duplicate-safe: max_index picks distinct
                # occurrences, and match_replace consumes the first-8)
                val16 = fin.tile([128, ncand], F32, tag="val16")
                posu = fin.tile([128, ncand], U32, tag="posu")
                nc.vector.max(val16[:, 0:8], cvals[tt][:])
                nc.vector.max_index(posu[:, 0:8], val16[:, 0:8], cvals[tt][:])
                if ncand > 8:
                    scratch = fin.tile([128, cw], F32, tag="scratch")
                    nc.vector.match_replace(
                        scratch[:], val16[:, 0:8], cvals[tt][:], -1e30)
                    nc.vector.max(val16[:, 8:16], scratch[:])
                    nc.vector.max_index(posu[:, 8:16], val16[:, 8:16],
                                        scratch[:])
                posf = fin.tile([128, ncand], F32, tag="posf")
                nc.vector.tensor_copy(posf[:], posu[:])
                # candidate ids: sum((iota == pos_m) * cidx) — iota is unique
                candf = fin.tile([128, ncand], F32, tag="candf")
                junk = fin.tile([128, cw], F32, tag="junk")
                for m in range(ncand):
                    nc.vector.scalar_tensor_tensor(
                        junk[:], iota_cw[:], posf[:, m:m + 1], cidx[tt][:],
                        op0=ALU.is_equal, op1=ALU.mult,
                        accum_out=candf[:, m:m + 1])
                candi = fin.tile([128, ncand], I32, tag="candi")
                nc.vector.tensor_copy(candi[:], candf[:])

                # Phase B: exact fp32 rescore of the candidates
                # (re-normalize this tile's targets in fp32 on the fly)
                traw = fin.tile([128, fdim], F32, tag="traw")
                nc.sync.dma_start(traw[:],
                                  t_in.ap()[tt * 128:(tt + 1) * 128, :])
                tss = fin.tile([128, 1], F32, tag="tss")
                tsq = fin.tile([128, fdim], F32, tag="tsq")
                nc.scalar.activation(tsq[:], traw[:], AF.Square,
                                     accum_out=tss[:])
                tnr = fin.tile([128, 1], F32, tag="tnr")
                nc.scalar.sqrt(tnr[:], tss[:])
                tiv = fin.tile([128, 1], F32, tag="tiv")
                nc.vector.reciprocal(tiv[:], tnr[:])
                tnb = fin.tile([128, fdim], F32, tag="tnb")
                nc.vector.tensor_scalar_mul(tnb[:], traw[:], tiv[:])
                exact = fin.tile([128, ncand], F32, tag="exact")
                if "phaseB" in ablate:
                    nc.vector.memset(exact[:], 0.0)
                for m in range(ncand) if "phaseB" not in ablate else []:
                    g = gat.tile([128, fdim], F32, tag="g")
                    nc.gpsimd.indirect_dma_start(
                        out=g[:], out_offset=None,
                        in_=s_in.ap(),
                        in_offset=bass.IndirectOffsetOnAxis(
                            ap=candi[:, m:m + 1], axis=0))
                    gb = gat.tile([128, 1], F32, tag="gb")
                    nc.gpsimd.indirect_dma_start(
                        out=gb[:], out_offset=None,
                        in_=impt_in.ap(),
                        in_offset=bass.IndirectOffsetOnAxis(
                            ap=candi[:, m:m + 1], axis=0))
                    gsq = gat.tile([128, fdim], F32, tag="gsq")
                    gss = gat.tile([128, 1], F32, tag="gss")
                    nc.scalar.activation(gsq[:], g[:], AF.Square,
                                         accum_out=gss[:])
                    gn = gat.tile([128, 1], F32, tag="gn")
                    nc.scalar.sqrt(gn[:], gss[:])
                    gi = gat.tile([128, 1], F32, tag="gi")
                    nc.vector.reciprocal(gi[:], gn[:])
                    dotc = gat.tile([128, 1], F32, tag="dotc")
                    gj = gat.tile([128, fdim], F32, tag="gj")
                    nc.vector.scalar_tensor_tensor(
                        gj[:], g[:], gi[:], tnb[:],
                        op0=ALU.mult, op1=ALU.mult, accum_out=dotc[:])
                    nc.vector.scalar_tensor_tensor(
                        exact[:, m:m + 1], gb[:],
                        W_IMPORTANCE, dotc[:], op0=ALU.mult, op1=ALU.add)

                # final top-8 on exact scores; ids via unique positions
                fvals = fin.tile([128, K], F32, tag="fvals")
                nc.vector.max(fvals[:], exact[:])
                fpos = fin.tile([128, K], U32, tag="fpos")
                nc.vector.max_index(fpos[:], fvals[:], exact[:])
                fposf = fin.tile([128, K], F32, tag="fposf")
                nc.vector.tensor_copy(fposf[:], fpos[:])
                gidx_f = fin.tile([128, K], F32, tag="gixf")
                junk2 = fin.tile([128, ncand], F32, tag="junk2")
                for k in range(K):
                    nc.vector.scalar_tensor_tensor(
                        junk2[:], iota_nc[:], fposf[:, k:k + 1], candf[:],
                        op0=ALU.is_equal, op1=ALU.mult,
                        accum_out=gidx_f[:, k:k + 1])
                gidx_i = fin.tile([128, K], I32, tag="gixi")
                nc.vector.tensor_copy(gidx_i[:], gidx_f[:])
                # softmax over the 8
                e = fin.tile([128, K], F32, tag="e")
                sume = fin.tile([128, 1], F32, tag="sume")
                nc.scalar.activation(e[:], fvals[:], AF.Exp, accum_out=sume[:])
                rse = fin.tile([128, 1], F32, tag="rse")
                nc.vector.reciprocal(rse[:], sume[:])
                alpha_t = fin.tile([128, K], F32, tag="al")
                nc.vector.tensor_scalar_mul(alpha_t[:], e[:], rse[:])

                rows = slice(tt * 128, (tt + 1) * 128)
                nc.sync.dma_start(idx_out.ap()[rows, :], gidx_i[:])
                nc.sync.dma_start(score_out.ap()[rows, :], fvals[:])
                nc.sync.dma_start(alpha_out.ap()[rows, :], alpha_t[:])

    nc.compile()
    return nc


_COMPILED = None


def _get_compiled():
    global _COMPILED
    if _COMPILED is None:
        _COMPILED = build_program()
    return _COMPILED


def kernel(target_edge_feats, source_edge_feats, source_importance,
           topk=8, chunk_size=4096):
    global LAST_RESULTS
    assert int(topk) == K
    t = np.ascontiguousarray(np.asarray(target_edge_feats, dtype=np.float32))
    s = np.ascontiguousarray(np.asarray(source_edge_feats, dtype=np.float32))
    imp = np.ascontiguousarray(
        np.asarray(source_importance, dtype=np.float32).reshape(1, -1))
    assert t.shape == (E_T, FDIM) and s.shape == (E_S, FDIM)

    nc = _get_compiled()
    in_maps = [
        {"t": t[i * T_LOCAL:(i + 1) * T_LOCAL], "s": s, "imp": imp,
         "impt": imp.reshape(-1, 1)}
        for i in range(N_CORES)
    ]
    res = run_bass_kernel_spmd(
        nc, in_maps, list(range(N_CORES)),
        trace=bool(os.environ.get("BASS_TRACE")))
    LAST_RESULTS = res
    idx = np.concatenate(
        [res.results[i]["idx"] for i in range(N_CORES)], axis=0)
    score = np.concatenate(
        [res.results[i]["score"] for i in range(N_CORES)], axis=0)
    alpha = np.concatenate(
        [res.results[i]["alpha"] for i in range(N_CORES)], axis=0)
    return idx.astype(np.int32), score.astype(np.float32), alpha.astype(np.float32)

